# revision 35
# baseline (speedup 1.0000x reference)
"""Grouped-Query Attention (B=2, T=2048, H=2048, 16 q-heads, 4 kv-heads, d=128,
causal) on 8 Trainium2 NeuronCores.

Sharding: core c = (batch b, kv-group g) with b = c // 4, g = c % 4.
Each core handles one batch element, one kv head, and its 4 q heads:
  - Q/K/V projections for its slice (tensor-parallel over heads)
  - causal attention for 4 q heads against the shared K/V head
  - partial o_proj (row-parallel): out_partial = O_heads @ w_o[:, cols].T
Host sums the 4 per-batch partials (the row-parallel all-reduce) and stacks.

Device layouts (chosen so no transposes are ever needed on-chip):
  QT, KT: [d=128, T]  (projection computed directly transposed)
  V:      [T-tile=128, d]
  scores: computed directly transposed as ST [k, q] via lhsT=KT_j, rhs=QT
  P = exp(ST/sqrt(d)) stays [k, q] and feeds PV as rhs -> OT [d, q] which is
  exactly the lhsT the o_proj needs.
All matmul inputs bf16, PSUM accumulation fp32, softmax in fp32.

Perf structure (baseline 254.8us -> this version):
  - softmax denominators no longer burn PE cycles per tile: P tiles are
    accumulated on the vector engine (bf16 in-place adds) and a single
    ones-matmul per (head, chunk) reduces the accumulated tile across
    partitions; 1/ls comes from vector reciprocal instead of scalar Ln+Exp.
    This removes ~29us of redundant broadcast matmul work from the PE.
  - projections are interleaved with attention per T-block
    (proj(t4) -> attention chunk qc=t4), which spreads the scalar-engine
    exp load across the whole timeline instead of packing it into the
    attention tail where it was ~90% busy.
  - dummy warm-up matmuls at t=0 keep the PE_HAM activity window alive so
    the clock gate (1.2 -> 2.4 GHz) releases before the first real matmul;
    baseline ran its first ~12 matmuls at half clock.
  - input DMAs are issued from two engine queues in parallel (sync: wk+x,
    gpsimd: wv/wq/msk/wo) so the first K-proj matmul starts ~3us earlier.
  - one continuous software pipeline across heads AND blocks: PV matmuls
    lag the S matmuls by `depth` tiles so the exp (+ mask) latency never
    stalls the in-order PE queue; a head's normalize chain (+ the previous
    chunk's o_proj slice as PE ballast) is deferred two flushes after its
    last tile so the PE never waits on the vector accumulator.
"""

import numpy as np
import ml_dtypes
from contextlib import ExitStack

import concourse.bass as bass
import concourse.mybir as mybir
import concourse.tile as tile
from concourse.bass_utils import run_bass_kernel_spmd

# ---------------------------------------------------------------------------
# Workaround for this compiler build's per-instruction sync-wait-slot limit
# (walrus setupSyncWait rejects >2 waits on an instruction). Post-process the
# serialized BIR: any instruction carrying more than 2 sem waits gets the
# excess moved onto injected same-engine Drain instructions placed directly
# before it (same queue, program order => identical semantics).
import json as _json

_WAIT_LIMITS = {}
_WAIT_LIMIT_DEFAULT = 1
# Opcodes whose same-engine-sem waits are redundant: these engines execute
# their queue in order and stream reads a full instruction behind the
# previous instruction's writes, so a wait on a semaphore incremented ONLY
# by earlier instructions of the same engine is always satisfied. (NOT
# Ldweights: the PE queue's reorder window can pull it ahead of in-flight
# matmuls, so its guards are real.)
_SELF_WAIT_DROP_OPCODES = {
    "Matmult", "Activation", "TensorTensor", "TensorCopy", "TensorScalarPtr",
    "TensorReduce", "Memset", "DMACopy", "DmaTransposeAnt", "Reciprocal",
}
_orig_to_json_bytes = bass.Bass.to_json_bytes


def _split_waits_json(bj: bytes) -> bytes:
    m = _json.loads(bj)
    ctr = 0
    changed = False
    # sem id -> set of engines that ever update it
    upd = {}
    for f in m["functions"]:
        for blk in f["blocks"]:
            for inst in blk["instructions"]:
                si = inst.get("sync_info") or {}
                for u in si.get("on_update") or []:
                    upd.setdefault(u["id"], set()).add(inst["engine"])
    for f in m["functions"]:
        for blk in f["blocks"]:
            out = []
            for inst in blk["instructions"]:
                si = inst.get("sync_info") or {}
                w = si.get("on_wait") or []
                if w and inst.get("opcode") in _SELF_WAIT_DROP_OPCODES:
                    w2 = [x for x in w
                          if not (x.get("wait_mode") == "sem-ge-imm"
                                  and upd.get(x["id"]) == {inst["engine"]})]
                    if len(w2) != len(w):
                        changed = True
                        si["on_wait"] = w = w2
                lim = _WAIT_LIMITS.get(inst.get("opcode"), _WAIT_LIMIT_DEFAULT)
                if len(w) > lim:
                    changed = True
                    extra, keep = w[:-lim], w[-lim:]
                    si["on_wait"] = keep
                    for i in range(0, len(extra), 1):
                        ctr += 1
                        out.append({
                            "debug": inst.get("debug", 0),
                            "engine": inst["engine"],
                            "ins": [],
                            "is_reset_sema": False,
                            "name": f"I-wsplit-{ctr}",
                            "opcode": "Drain",
                            "outs": [],
                            "sync_info": {
                                "on_update": [],
                                "on_wait": extra[i:i + 1],
                            },
                        })
                out.append(inst)
            if changed:
                blk["instructions"] = out
    if not changed:
        return bj
    return _json.dumps(m).encode()


def _to_json_bytes_patched(self, *a, **k):
    return _split_waits_json(_orig_to_json_bytes(self, *a, **k))


bass.Bass.to_json_bytes = _to_json_bytes_patched
# ---------------------------------------------------------------------------

HIDDEN = 2048
N_HEADS = 16
N_KV = 4
HD = 128
B, T = 2, 2048
G = N_HEADS // N_KV          # q heads per core = 4
HC = HIDDEN // 128           # contraction chunks = 16
NCORES = 8
SCALE = HD ** -0.5
NWARM = 16                   # HAM warm-up matmuls

BF16 = mybir.dt.bfloat16
F32 = mybir.dt.float32

_CACHE = {}
LAST_RESULTS = None


def _build_program():
    nc = bass.Bass("TRN2")
    # host-repacked inputs: partition dim first, long contiguous rows
    xb = nc.dram_tensor("xb", [128, 4, HC, 512], BF16, kind="ExternalInput")
    wq = nc.dram_tensor("wq", [128, G, HC, HD], BF16, kind="ExternalInput")
    wk = nc.dram_tensor("wk", [128, HC, HD], BF16, kind="ExternalInput")
    wv = nc.dram_tensor("wv", [128, HC, HD], BF16, kind="ExternalInput")
    wo = nc.dram_tensor("wo", [128, G, HIDDEN], BF16, kind="ExternalInput")
    msk = nc.dram_tensor("msk", [128, 128], BF16, kind="ExternalInput")
    out = nc.dram_tensor("out", [T, HIDDEN], BF16, kind="ExternalOutput")

    EXP = mybir.ActivationFunctionType.Exp

    with tile.TileContext(nc) as tc, ExitStack() as ctx:
        sing = ctx.enter_context(tc.tile_pool(name="sing", bufs=1))
        ptp = ctx.enter_context(tc.tile_pool(name="ptp", bufs=24))
        vecp = ctx.enter_context(tc.tile_pool(name="vecp", bufs=3))
        accp = ctx.enter_context(tc.tile_pool(name="accp", bufs=3))
        otnp = ctx.enter_context(tc.tile_pool(name="otnp", bufs=8))
        outp = ctx.enter_context(tc.tile_pool(name="outp", bufs=3))
        psum = ctx.enter_context(tc.tile_pool(name="psum", bufs=2, space="PSUM"))

        xT_sb = sing.tile([128, 4, HC, 512], BF16)
        wq_sb = sing.tile([128, G, HC, HD], BF16)
        wk_sb = sing.tile([128, HC, HD], BF16)
        wv_sb = sing.tile([128, HC, HD], BF16)
        wo_sb = sing.tile([128, G, HIDDEN], BF16)
        msk_sb = sing.tile([128, 128], BF16)
        ones_sb = sing.tile([128, 128], BF16)
        gpde_sb = sing.tile([128, 512], BF16)
        qt_sb = sing.tile([128, G, T], BF16)
        kt_sb = sing.tile([128, T], BF16)
        vt_sb = sing.tile([128, T], BF16)
        v_sb = sing.tile([128, HC, HD], BF16)

        nc.vector.memset(ones_sb, 1.0)

        # --- PE warm-up: keep the HAM activity window alive from t=0 so the
        # clock gate releases (1.2 -> 2.4 GHz) before the first real matmul.
        for i in range(NWARM):
            wp = psum.tile([128, 512], F32, tag="op", bufs=2, name=f"warm_{i}")
            nc.tensor.matmul(wp[:, 0:128], lhsT=ones_sb, rhs=ones_sb,
                             start=True, stop=True)

        # --- input DMAs: two issue rings in parallel, ordered by need time.
        # ring A (sync): wk + x blocks 0-1; ring B (gpsimd): per-head wq, wv,
        # msk, wo, x blocks 2-3. Each ring gets ~half the DMA engines, so
        # spreading late-need transfers (wo, xb2/3) keeps early ones fast.
        nc.sync.dma_start(out=wk_sb, in_=wk[:, :, :])
        for qtr in range(4):
            nc.sync.dma_start(out=xT_sb[:, 0, 4 * qtr:4 * qtr + 4],
                              in_=xb[:, 0, 4 * qtr:4 * qtr + 4])
        for t4 in range(1, 4):
            nc.sync.dma_start(out=xT_sb[:, t4], in_=xb[:, t4])
        nc.sync.dma_start(out=wo_sb, in_=wo[:, :, :])
        # stagger ring B so ring A's critical wk+x0 transfers get the full
        # DMA bandwidth for the first ~4us (gpsimd busy-work as the delay)
        for i in range(7):
            nc.gpsimd.memset(gpde_sb, 0.0)
        nc.gpsimd.dma_start(out=wq_sb[:, 0], in_=wq[:, 0])
        nc.gpsimd.dma_start(out=wv_sb, in_=wv[:, :, :])
        for h in range(1, G):
            nc.gpsimd.dma_start(out=wq_sb[:, h], in_=wq[:, h])
        nc.gpsimd.dma_start(out=msk_sb, in_=msk[:, :])

        # ---- software-pipelined attention state ----
        otns = {}
        pend = []
        hooks = []  # [flushes_remaining, fn]

        def emit_oproj(qc, tt, split_casts=False):
            stage = outp.tile([128, HIDDEN], BF16, tag="stage", bufs=3,
                              name=f"stage_{qc}_{tt}")
            for ec in range(4):
                op = psum.tile([128, 512], F32, tag="op", bufs=2,
                               name=f"op_{qc}_{tt}_{ec}")
                for h in range(G):
                    nc.tensor.matmul(
                        op, lhsT=otns[(qc, h)][:, tt * 128:(tt + 1) * 128],
                        rhs=wo_sb[:, h, ec * 512:(ec + 1) * 512],
                        start=(h == 0), stop=(h == G - 1),
                    )
                esl = slice(ec * 512, (ec + 1) * 512)
                if split_casts and ec % 2 == 1:
                    nc.scalar.copy(stage[:, esl], op)
                else:
                    nc.vector.tensor_copy(stage[:, esl], op)
                r0 = qc * 512 + tt * 128
                if split_casts:
                    # tail: one DMA per ec slice so the last exposed
                    # transfer is only a quarter tile
                    nc.sync.dma_start(out=out[r0:r0 + 128, esl],
                                      in_=stage[:, esl])
                elif ec == 1:
                    nc.sync.dma_start(out=out[r0:r0 + 128, 0:1024],
                                      in_=stage[:, 0:1024])
            if not split_casts:
                r0 = qc * 512 + tt * 128
                nc.sync.dma_start(out=out[r0:r0 + 128, 1024:2048],
                                  in_=stage[:, 1024:2048])

        def flush_one():
            ot_, jj, pp, off, q0, first, last, fin, acc_ = pend.pop(0)
            sgc = not (first and q0 == 0)
            nc.tensor.matmul(ot_[:, q0:512], lhsT=v_sb[:, jj, :],
                             rhs=pp[:, off:off + 512 - q0],
                             start=first, stop=last, skip_group_check=sgc)
            # accumulate P into the softmax-denominator tile (vector engine)
            if first:
                nc.vector.tensor_copy(acc_, pp[:, off:off + 512])
            else:
                nc.vector.tensor_add(acc_[:, q0:512], acc_[:, q0:512],
                                     pp[:, off:off + 512 - q0])
            for hk in hooks:
                hk[0] -= 1
            while hooks and hooks[0][0] <= 0:
                hooks.pop(0)[1]()
            if last and fin is not None:
                # defer the normalize chain so its ones-matmul (which waits
                # on the vector accumulator) never stalls the in-order PE
                hooks.append([2, fin])

        def make_fin(qc, h, ot_, acc_, prev_qc):
            # two deferred stages: the ones-matmul + 1/ls chain (scalar Ln
            # then Exp(-x); DVE reciprocal is an iterative op, ~5x slower)
            # first, then -- three flushes of covering PE work later -- the
            # normalize mul and the o_proj ballast whose matmuls wait on it.
            def fin_a():
                lsd = psum.tile([128, 512], F32, tag="op", bufs=2,
                                name=f"lsd_{qc}_{h}")
                nc.tensor.matmul(lsd, lhsT=ones_sb, rhs=acc_,
                                 start=True, stop=True)
                lnl = vecp.tile([128, 512], F32, tag="lnl", bufs=3,
                                name=f"lnl_{qc}_{h}")
                nc.scalar.activation(lnl, lsd,
                                     mybir.ActivationFunctionType.Ln)
                rec = vecp.tile([128, 512], F32, tag="rec", bufs=3,
                                name=f"rec_{qc}_{h}")
                nc.scalar.activation(rec, lnl,
                                     mybir.ActivationFunctionType.Exp,
                                     scale=-1.0)

                def fin_b():
                    otn = otnp.tile([128, 512], BF16, tag="otn", bufs=8,
                                    name=f"otn_{qc}_{h}")
                    nc.vector.tensor_mul(otn, ot_, rec)
                    otns[(qc, h)] = otn
                    # interleave the previously-processed chunk's o_proj
                    if prev_qc is not None:
                        emit_oproj(prev_qc, h)
                hooks.append([3, fin_b])
            return fin_a

        # ---- interleaved blocks: proj(t4) then attention chunk qc=t4 ----
        for t4 in range(4):
            tsl = slice(t4 * 512, (t4 + 1) * 512)
            # Projections, interleaved at xb-quarter-chunk granularity so the
            # PE tracks DMA arrival in block 0 (and the HAM window never
            # dips): K and Q0 run at chunk pace, V and Q1-3 lag one chunk,
            # so the kt/qt0 copies land just before the attention chunk's
            # first S matmuls need them. All six accumulations live in PSUM
            # at once: kp/vtp in one st-pair tile, qp0/qp1 in another,
            # qp2/qp3 on the ot tag.
            stp1 = psum.tile([128, 1024], F32, tag="st", bufs=2,
                             name=f"pkv_{t4}")
            stp2 = psum.tile([128, 1024], F32, tag="st", bufs=2,
                             name=f"pq01_{t4}")
            kp, vtp = stp1[:, 0:512], stp2[:, 0:512]
            qps = [stp1[:, 512:1024], stp2[:, 512:1024],
                   psum.tile([128, 512], F32, tag="ot", bufs=2,
                             name=f"pq2_{t4}"),
                   psum.tile([128, 512], F32, tag="ot", bufs=2,
                             name=f"pq3_{t4}")]

            def pmm(dst, lhsT, c):
                nc.tensor.matmul(dst, lhsT=lhsT, rhs=xT_sb[:, t4, c, :],
                                 start=(c == 0), stop=(c == HC - 1),
                                 skip_group_check=True)

            for q4 in range(5):
                if q4 < 4:
                    for c in range(4 * q4, 4 * q4 + 4):
                        pmm(kp, wk_sb[:, c, :], c)
                    for c in range(4 * q4, 4 * q4 + 4):
                        pmm(qps[0], wq_sb[:, 0, c, :], c)
                    if q4 == 3:
                        nc.scalar.copy(kt_sb[:, tsl], kp)
                        nc.scalar.copy(qt_sb[:, 0, tsl], qps[0])
                if q4 > 0:
                    for c in range(4 * q4 - 4, 4 * q4):
                        pmm(vtp, wv_sb[:, c, :], c)
                    for h in range(1, G):
                        for c in range(4 * q4 - 4, 4 * q4):
                            pmm(qps[h], wq_sb[:, h, c, :], c)
                    if q4 == 4:
                        nc.scalar.copy(vt_sb[:, tsl], vtp)
                        for ts in range(4):
                            tt = 4 * t4 + ts
                            nc.sync.dma_start_transpose(
                                out=v_sb[:, tt, :],
                                in_=vt_sb[:, tt * 128:(tt + 1) * 128])
                        for h in range(1, G):
                            nc.scalar.copy(qt_sb[:, h, tsl], qps[h])

            # ---- attention chunk qc = t4 (needs only K/Q/V of t4' <= t4) ----
            qc = t4
            njt = 4 * qc + 4
            # off-diagonal j-tiles first: their K tiles come from earlier
            # blocks, so attention starts before this block's kt/vt copies
            # and V transposes have landed (diagonal tiles run last). The
            # first tile is always full-width (q0=0), as the PSUM
            # accumulation start requires.
            js = list(range(0, 4 * qc)) + list(range(4 * qc, njt))
            depth = min(njt - 1, 6)
            for h in range(G):
                ot = psum.tile([128, 512], F32, tag="ot", bufs=2,
                               name=f"ot_{qc}_{h}")
                acc = accp.tile([128, 512], BF16, tag="acc", bufs=3,
                                name=f"acc_{qc}_{h}")
                fin = make_fin(qc, h, ot, acc,
                               qc - 1 if qc >= 1 else None)
                # two j-tiles share one [128,1024] score tile, packed
                # contiguously (each matmul region stays within a PSUM bank),
                # so ONE exp covers both -- halves the scalar instruction
                # count and the number of PE-waits-on-exp events
                for pidx in range(njt // 2):
                    jA, jB = js[2 * pidx], js[2 * pidx + 1]
                    stp = psum.tile([128, 1024], F32, tag="st", bufs=2,
                                    name=f"st_{qc}_{h}_{pidx}")
                    ptt = ptp.tile([128, 1024], BF16, tag="pt", bufs=12,
                                   name=f"pt_{qc}_{h}_{pidx}")
                    off = 0
                    ents = []
                    for j in (jA, jB):
                        # exact causal: diagonal tile s only feeds queries
                        # q >= 128*s within this 512-wide q chunk
                        s = j - 4 * qc
                        q0 = 128 * s if s >= 0 else 0
                        width = 512 - q0
                        nc.tensor.matmul(
                            stp[:, off:off + width],
                            lhsT=kt_sb[:, j * 128:(j + 1) * 128],
                            rhs=qt_sb[:, h, qc * 512 + q0:(qc + 1) * 512],
                            start=True, stop=True,
                            skip_group_check=(j == jB),
                        )
                        ents.append((j, q0, off, width))
                        off += width
                    nc.scalar.activation(ptt[:, 0:off], stp[:, 0:off],
                                         EXP, scale=float(SCALE))
                    for ei, (j, q0, o_, width) in enumerate(ents):
                        s = j - 4 * qc
                        if s >= 0:
                            # triangle mask on the q==key diagonal subtile
                            nc.vector.tensor_mul(ptt[:, o_:o_ + 128],
                                                 ptt[:, o_:o_ + 128],
                                                 msk_sb[:, 0:128])
                        idx = 2 * pidx + ei
                        pend.append((ot, j, ptt, o_, q0,
                                     idx == 0, idx == njt - 1,
                                     fin if idx == njt - 1 else None, acc))
                        if len(pend) > depth:
                            flush_one()
            # drain the chunk's pipeline before the next block's projections:
            # the proj matmuls (no exp dependency) are the PE cover, and the
            # last head's normalize chain isn't stranded behind them in the
            # scalar queue.
            while pend:
                flush_one()
            while hooks:
                hooks.pop(0)[1]()
        for tt in range(4):
            emit_oproj(3, tt, split_casts=True)
    return nc


def _masks():
    kl = np.arange(128)[:, None]
    ql = np.arange(128)[None, :]
    return (kl <= ql).astype(ml_dtypes.bfloat16)


def kernel(x, w_q, w_kv, w_o):
    global LAST_RESULTS
    if "nc" not in _CACHE:
        _CACHE["nc"] = _build_program()
        _CACHE["msk"] = _masks()
    nc = _CACHE["nc"]
    bf = ml_dtypes.bfloat16
    x = np.asarray(x, dtype=np.float32)
    w_q = np.asarray(w_q, dtype=np.float32)
    w_kv = np.asarray(w_kv, dtype=np.float32)
    w_o = np.asarray(w_o, dtype=np.float32)

    in_maps = []
    for c in range(NCORES):
        b, g = c // 4, c % 4
        # x[b]: [T, H] -> [p, t4, c, t]
        xbh = np.ascontiguousarray(
            x[b].reshape(4, 512, HC, 128).transpose(3, 0, 2, 1)).astype(bf)
        # w_q rows for this core's 4 heads: [512, H] -> [p, h, c, m]
        # (per-head contiguous so each head's weights are one clean DMA)
        wqg = np.ascontiguousarray(
            w_q[512 * g:512 * (g + 1), :].reshape(G, 128, HC, 128)
            .transpose(3, 0, 2, 1)).astype(bf)
        wkg = np.ascontiguousarray(
            w_kv[128 * g:128 * (g + 1), :].T.reshape(HC, 128, 128)
            .transpose(1, 0, 2)).astype(bf)
        wvg = np.ascontiguousarray(
            w_kv[512 + 128 * g:512 + 128 * (g + 1), :].T.reshape(HC, 128, 128)
            .transpose(1, 0, 2)).astype(bf)
        # w_o cols for this core's heads: [H, 512] -> [p(d), h, e]
        wog = np.ascontiguousarray(
            w_o[:, 512 * g:512 * (g + 1)].T.reshape(G, 128, HIDDEN)
            .transpose(1, 0, 2)).astype(bf)
        in_maps.append({
            "xb": xbh, "wq": wqg, "wk": wkg, "wv": wvg, "wo": wog,
            "msk": _CACHE["msk"],
        })

    res = run_bass_kernel_spmd(nc, in_maps, core_ids=list(range(NCORES)))
    LAST_RESULTS = res
    outs = res.results
    o = [outs[c]["out"].astype(np.float32) for c in range(NCORES)]
    out = np.stack([o[0] + o[1] + o[2] + o[3], o[4] + o[5] + o[6] + o[7]])
    return out
